# revision 16
# baseline (speedup 1.0000x reference)
"""Trainium2 Bass kernel for the segment_reduce loss (nn_Loss_65996467471179).

Strategy (data-parallel over curves):
  - C=65536 curves of L=256 points. Shard curves across 8 cores (8192 each).
  - Each core streams its 5 big arrays (An, A_r, Ac, Aj, Ap) once from HBM in
    [128, 2048] chunks (8 curves per partition), computes all per-curve and
    global partial reductions on-chip, and writes a small [128, 272] float32
    accumulator block back to DRAM.
  - Ci is only read at end-of-curve indices; that gather plus all C-length /
    O(4)-length pure-input terms (correlation moments, Rd25/dHa/Topt sign
    penalties) are folded on the host, which also combines the 8 cores'
    partial blocks into the final scalar in float64.

Per-curve math on device (curve rows live along the free axis, 8 per
partition):
  Acj   = Ac - Aj
  A     = |Acj|
  mn    = min_l A                      (segmented 3D reduce)
  sAcj  = sum_l Acj, sAbs = sum_l A    (ls_Ac = (sAbs+sAcj)/2, ls_Aj = (sAbs-sAcj)/2)
  gint  = sum_l (A == mn) * (1.1*Aj - Ap)   == 1.1*Aj[argmin] - Ap[argmin]
  plus global sums of (An-A_r)^2 and relu(-Ap), and end-of-curve columns.
"""

import os
import sys

import numpy as np

sys.path.insert(0, "/opt/trn_rl_repo")

import concourse.bass as bass
import concourse.bacc as bacc
import concourse.tile as tile
from concourse import mybir
from concourse.bass_utils import run_bass_kernel_spmd
from contextlib import ExitStack

NCORES = 8
C = 65536
L = 256
N = C * L
S = C // NCORES          # curves per core
NSH = S * L              # elements per core per big array
P = 128                  # partitions
F = 2048                 # elements per partition per chunk
J = F // L               # curves per partition per chunk
M = NSH // (P * F)       # chunks per core (8)
NCOL = M * J             # per-curve accumulator columns (64)

KELVIN = 273.15
FIT_AP_CI = 500.0
TARGET_R = 0.7

f32 = mybir.dt.float32

# accumulator block column layout
MSE0 = 0            # [M]  per-chunk per-partition sum (An-A_r)^2
APN0 = MSE0 + M     # [M]  per-chunk per-partition sum relu(-Ap)
P30 = APN0 + M      # [NCOL] relu(3*gint) per curve
LS0 = P30 + NCOL    # [NCOL] w*(relu(8-ls_Aj)+relu(8-ls_Ac)) per curve
E10 = LS0 + NCOL    # [NCOL] relu(Ap_end-Aj_end)*fitw per curve
E20 = E10 + NCOL    # [NCOL] relu(Aj_end-Ac_end) per curve
ACCW = E20 + NCOL   # 272


def _build_kernel(reps=None):
    """reps=None: normal single-pass kernel. reps=R: wrap the whole body in a
    runtime For_i loop executing it R times (for HW timing via slope)."""
    OP = mybir.AluOpType
    AF = mybir.ActivationFunctionType
    AX = mybir.AxisListType

    nc = bacc.Bacc("TRN2", target_bir_lowering=False, debug=False, num_devices=NCORES)
    big = {
        nm: nc.declare_dram_parameter(nm, [NSH], f32, isOutput=False)
        for nm in ("An", "Ar", "Ac", "Aj", "Ap")
    }
    wdev = nc.declare_dram_parameter("wdev", [P, NCOL], f32, isOutput=False)
    fitw = nc.declare_dram_parameter("fitw", [P, NCOL], f32, isOutput=False)
    acc = nc.declare_dram_parameter("acc", [P, ACCW], f32, isOutput=True)

    with ExitStack() as ctx:
        tc = ctx.enter_context(tile.TileContext(nc))
        inp = ctx.enter_context(tc.tile_pool(name="inp", bufs=2))
        wrk = ctx.enter_context(tc.tile_pool(name="wrk", bufs=2))
        per = ctx.enter_context(tc.tile_pool(name="per", bufs=1))

        accT = per.tile([P, ACCW], f32, tag="accT")
        mnB = per.tile([P, NCOL], f32, tag="mnB")
        sAcj = per.tile([P, NCOL], f32, tag="sAcj")
        sAbs = per.tile([P, NCOL], f32, tag="sAbs")
        gint = per.tile([P, NCOL], f32, tag="gint")
        eAp = per.tile([P, NCOL], f32, tag="eAp")
        eAj = per.tile([P, NCOL], f32, tag="eAj")
        eAc = per.tile([P, NCOL], f32, tag="eAc")
        wT = per.tile([P, NCOL], f32, tag="wT")
        fT = per.tile([P, NCOL], f32, tag="fT")
        junkD = per.tile([P, L], f32, tag="junkD")
        junkA = per.tile([P, F], f32, tag="junkA")
        t1 = per.tile([P, NCOL], f32, tag="t1")
        t2 = per.tile([P, NCOL], f32, tag="t2")
        r1 = per.tile([P, NCOL], f32, tag="r1")
        r2 = per.tile([P, NCOL], f32, tag="r2")
        b8 = per.tile([P, 1], f32, tag="b8")
        nc.vector.memset(b8, 8.0)

        nc.sync.dma_start(out=wT, in_=wdev[:])
        nc.sync.dma_start(out=fT, in_=fitw[:])

        def body():
            _trace_body(nc, tc, big, acc, inp, wrk, accT, mnB, sAcj, sAbs, gint,
                        eAp, eAj, eAc, wT, fT, junkD, junkA, t1, t2, r1, r2, b8)

        if reps is None:
            body()
        else:
            with tc.For_i(0, reps, 1):
                body()

    nc.compile()
    return nc


def _trace_body(nc, tc, big, acc, inp, wrk, accT, mnB, sAcj, sAbs, gint,
                eAp, eAj, eAc, wT, fT, junkD, junkA, t1, t2, r1, r2, b8):
    OP = mybir.AluOpType
    AF = mybir.ActivationFunctionType
    AX = mybir.AxisListType
    if True:
        for m in range(M):
            t = {}
            for nm in ("An", "Ar", "Ac", "Aj", "Ap"):
                t[nm] = inp.tile([P, F], f32, tag=nm, name=f"in_{nm}_{m}")
                src = big[nm][:].rearrange("(m p f) -> m p f", m=M, p=P, f=F)[m]
                nc.sync.dma_start(out=t[nm], in_=src)

            cols = slice(m * J, (m + 1) * J)

            # --- GPSIMD: the two elementwise 2-input streams + end copies ---
            d = wrk.tile([P, F], f32, tag="d")
            nc.gpsimd.tensor_tensor(out=d, in0=t["An"], in1=t["Ar"], op=OP.subtract)
            G = wrk.tile([P, F], f32, tag="G")
            nc.vector.scalar_tensor_tensor(
                out=G, in0=t["Aj"], scalar=1.1, in1=t["Ap"],
                op0=OP.mult, op1=OP.subtract,
            )
            for nm, dst in (("Ap", eAp), ("Aj", eAj), ("Ac", eAc)):
                ends = t[nm].rearrange("p (j l) -> p j l", l=L)[:, :, L - 1 : L]
                nc.gpsimd.tensor_copy(out=dst[:, cols], in_=ends)

            # --- ACT: global accumulations + abs ---
            nc.scalar.activation(
                out=junkA, in_=d, func=AF.Square,
                accum_out=accT[:, MSE0 + m : MSE0 + m + 1],
            )
            nc.scalar.activation(
                out=junkA, in_=t["Ap"], func=AF.Relu, scale=-1.0,
                accum_out=accT[:, APN0 + m : APN0 + m + 1],
            )
            Acj = wrk.tile([P, F], f32, tag="Acj")
            nc.gpsimd.tensor_tensor(out=Acj, in0=t["Ac"], in1=t["Aj"], op=OP.subtract)
            A = wrk.tile([P, F], f32, tag="A")
            nc.scalar.activation(out=A, in_=Acj, func=AF.Abs)

            # --- DVE: segmented per-curve reduces + argmin-select ---
            Acj3 = Acj.rearrange("p (j l) -> p j l", l=L)
            A3 = A.rearrange("p (j l) -> p j l", l=L)
            nc.vector.tensor_reduce(out=mnB[:, cols], in_=A3, axis=AX.X, op=OP.min)
            nc.vector.tensor_reduce(out=sAcj[:, cols], in_=Acj3, axis=AX.X, op=OP.add)
            nc.vector.tensor_reduce(out=sAbs[:, cols], in_=A3, axis=AX.X, op=OP.add)
            for j in range(J):
                c = m * J + j
                nc.vector.scalar_tensor_tensor(
                    out=junkD,
                    in0=A[:, j * L : (j + 1) * L],
                    scalar=mnB[:, c : c + 1],
                    in1=G[:, j * L : (j + 1) * L],
                    op0=OP.is_equal,
                    op1=OP.mult,
                    accum_out=gint[:, c : c + 1],
                )

        # --- post-loop epilogue on [128, 64] blocks ---
        # ls penalty: relu(8 - ls_Aj) + relu(8 - ls_Ac), ls_* = (sAbs -+ sAcj)/2
        nc.vector.tensor_tensor(out=t1, in0=sAbs, in1=sAcj, op=OP.add)
        nc.scalar.activation(out=r1, in_=t1, func=AF.Relu, scale=-0.5, bias=b8)
        nc.vector.tensor_tensor(out=t2, in0=sAbs, in1=sAcj, op=OP.subtract)
        nc.scalar.activation(out=r2, in_=t2, func=AF.Relu, scale=-0.5, bias=b8)
        nc.vector.tensor_tensor(out=t1, in0=r1, in1=r2, op=OP.add)
        nc.vector.tensor_tensor(out=accT[:, LS0 : LS0 + NCOL], in0=t1, in1=wT, op=OP.mult)
        # crossover penalty: 3*relu(gint) == relu(3*gint)
        nc.scalar.activation(out=accT[:, P30 : P30 + NCOL], in_=gint, func=AF.Relu, scale=3.0)
        # end-of-curve penalties
        nc.vector.tensor_tensor(out=t2, in0=eAp, in1=eAj, op=OP.subtract)
        nc.scalar.activation(out=r1, in_=t2, func=AF.Relu)
        nc.vector.tensor_tensor(out=accT[:, E10 : E10 + NCOL], in0=r1, in1=fT, op=OP.mult)
        nc.vector.tensor_tensor(out=t2, in0=eAj, in1=eAc, op=OP.subtract)
        nc.scalar.activation(out=accT[:, E20 : E20 + NCOL], in_=t2, func=AF.Relu)

        nc.sync.dma_start(out=acc[:], in_=accT)


_NC_CACHE = {}
LAST_RESULTS = None


def _get_nc(reps=None):
    if reps not in _NC_CACHE:
        _NC_CACHE[reps] = _build_kernel(reps)
    return _NC_CACHE[reps]


def _curve_layout(x_per_curve: np.ndarray) -> np.ndarray:
    """Map a per-curve [S] array for one core into the device [P, NCOL] layout:
    dev[p, m*J + j] corresponds to curve m*(P*J) + p*J + j."""
    return np.ascontiguousarray(
        x_per_curve.reshape(M, P, J).transpose(1, 0, 2).reshape(P, NCOL)
    )


def prep_in_maps(An_o, Ac_o, Aj_o, Ap_o, A_r, Ci, mask_lightresp):
    w_full = (mask_lightresp == 0).astype(np.float32)        # [C]
    Ci_end = np.ascontiguousarray(Ci[L - 1 :: L])            # [C]
    fit_full = ((Ci_end > FIT_AP_CI).astype(np.float32) * w_full)  # [C]

    in_maps = []
    for k in range(NCORES):
        cur = slice(k * S, (k + 1) * S)
        el = slice(k * NSH, (k + 1) * NSH)
        in_maps.append({
            "An": np.ascontiguousarray(An_o[el]),
            "Ar": np.ascontiguousarray(A_r[el]),
            "Ac": np.ascontiguousarray(Ac_o[el]),
            "Aj": np.ascontiguousarray(Aj_o[el]),
            "Ap": np.ascontiguousarray(Ap_o[el]),
            "wdev": _curve_layout(w_full[cur]),
            "fitw": _curve_layout(fit_full[cur]),
        })
    return in_maps


def kernel(An_o, Ac_o, Aj_o, Ap_o, A_r, Ci, Vcmax25, Jmax25, Rd25,
           dHa_Vcmax, dHa_Jmax, dHa_TPU, Topt_Vcmax, Topt_Jmax, Topt_TPU,
           mask_lightresp):
    nc = _get_nc()
    in_maps = prep_in_maps(An_o, Ac_o, Aj_o, Ap_o, A_r, Ci, mask_lightresp)

    res = run_bass_kernel_spmd(
        nc, in_maps, core_ids=list(range(NCORES)),
        trace=bool(int(os.environ.get("KERNEL_TRACE", "0"))),
    )
    global LAST_RESULTS
    LAST_RESULTS = res
    blocks = [r["acc"].astype(np.float64) for r in res.results]

    mse = sum(b[:, MSE0 : MSE0 + M].sum() for b in blocks)
    apn = sum(b[:, APN0 : APN0 + M].sum() for b in blocks)
    p3 = sum(b[:, P30 : P30 + NCOL].sum() for b in blocks)
    ls = sum(b[:, LS0 : LS0 + NCOL].sum() for b in blocks)
    e1 = sum(b[:, E10 : E10 + NCOL].sum() for b in blocks)
    e2 = sum(b[:, E20 : E20 + NCOL].sum() for b in blocks)

    # host-side terms (tiny inputs only)
    w = (mask_lightresp == 0).astype(np.float64)
    x = Jmax25.astype(np.float64)
    y = Vcmax25.astype(np.float64)
    nw = w.sum()
    if nw > 0:
        my = (w * y).sum() / nw
        mx = (w * x).sum() / nw
        vy = (y - my) * w
        vx = (x - mx) * w
        denom = np.sqrt((vx * vx).sum()) * np.sqrt((vy * vy).sum())
        cost = (vx * vy).sum() / denom if denom != 0.0 else np.nan
    else:
        cost = np.nan
    if np.isnan(cost):
        cost = 0.0
    cost = min(cost, TARGET_R)

    relu = lambda v: np.maximum(v, 0.0)
    loss = mse * 10.0 / N
    loss += TARGET_R - cost
    loss += relu(-Rd25.astype(np.float64)).sum()
    loss += relu(-dHa_Vcmax.astype(np.float64)).sum() * 10.0
    loss += relu(-dHa_Jmax.astype(np.float64)).sum()
    loss += relu(-dHa_TPU.astype(np.float64)).sum()
    loss += relu(KELVIN - Topt_Vcmax.astype(np.float64)).sum()
    loss += relu(KELVIN - Topt_Jmax.astype(np.float64)).sum()
    loss += relu(KELVIN - Topt_TPU.astype(np.float64)).sum()
    loss += apn
    loss += e1 * 0.15
    loss += e2
    loss += p3
    loss += ls

    return np.float32(loss)
